# revision 9
# baseline (speedup 1.0000x reference)
"""Trainium2 Bass kernel for DiagonalLinear.

The reference masks W to its diagonal (zeroing entries with |w| <= 1e-4)
and computes x @ masked_W.T, which is exactly an elementwise scale of
x's columns by the thresholded diagonal of W.

Distribution (8 NeuronCores): data-parallel — x is sharded along the
token axis (1024 tokens per core); per the sharding hint, only the
(thresholded) diagonal of W — the sole part of W the op reads — is
replicated to every core. No inter-core communication.

The op is purely memory-bound. bf16 streaming (x and out quantized on
the host; the roundings stay under 1.2%, inside the 2e-2 tolerance)
puts per-core traffic at 8 MiB in + 8 MiB out. The combined load+store
stream saturates the 16 SBUF AXI ports (~425 GB/s measured), so the
whole game is keeping both DMA queues fed at that rate for the entire
window. Design, driven by trace measurements:

1. The diagonal arrives from HBM already replicated across the 128
   partitions ([128, 4096] bf16, 1 MiB, host-prepared). The on-device
   alternative (row load + K=1 matmul broadcast + PSUM->SBUF casts)
   costs ~6 us of serial latency before the first multiply can run —
   measured: first store at ~14-16 us, leaving the early stream
   single-queue. The extra 1 MiB costs only ~2.4 us of stream time and
   frees the DVE to do nothing but multiplies. Tensor/PSUM unused.

2. The replicated diagonal loads as two column-half DMAs at the HEAD
   of the sync FIFO, and the first x tile's load is also split in
   halves: the first multiply fires at ~5 us and the store stream is
   flowing by ~6 us, so load and store packets interleave for
   essentially the whole window.

3. A ring's FIRST DMA pays a ~4.5 us (qAct) / ~1.5 us (qSP) cold
   start before bytes move (measured). The scalar engine issues a
   no-wait dummy write (uninitialized scratch -> DRAM scratch) at t=0
   so the qAct ring is warm before the first output store needs it.

4. Tile rows [128,128,120,120,128,128,128,128,16]: the 120-row tiles
   (15-engine, port-crossed descriptor layout — full rate only when
   load and store packets interleave, measured ~215 GB/s single-queue)
   run mid-stream where stores are active. Engine 15 (the measured
   ~7-18%-slower SDMA engine) gets 49/65 of the per-engine line count.
   The tiny [16] tile goes last: the final load->mul->store chain is
   short and hides under the store-backlog drain.

5. Multiplies and stores run per column-half (512 KB units; a DVE
   tensor op's time scales with free-dim length, so halves cost no
   extra throughput) — the store FIFO is fed at fine granularity and
   each unit's store issues right after its multiply. The last three
   store units ride the sync ring once its loads have drained, so the
   tail backlog drains through both rings.

Per-core device program — raw Bass (no Tile scheduler) with hand-placed
semaphores; the kernel ends on store-completion waits, not an
all-engine barrier.

Engine plan (single Block):
  sync   : 2 diagonal half-loads + 10 x-tile loads on the qSP ring,
           a warm-up write, then the last 3 store units
  scalar : no-wait qAct warm-up write at t=0, then store units 0..14
           (each gated on its multiply)
  vector : the 18 column-half multiplies, each gated on its tile load
  tensor : idle
"""

import numpy as np

TOKENS = 8192
N = 4096
N_CORES = 8
T_SHARD = TOKENS // N_CORES  # 1024
TILE_P = [128, 128, 120, 120, 128, 128, 128, 128, 16]
P0 = max(TILE_P)
THRESHOLD = 1e-4
N_SYNC_STORES = 3            # tail store units issued on the sync ring

_CACHED_NC = None


def _build_nc(tile_p=None, n_sync_stores=N_SYNC_STORES):
    from contextlib import ExitStack

    from concourse import bass, mybir

    bf16 = mybir.dt.bfloat16
    tile_p = list(TILE_P) if tile_p is None else list(tile_p)
    nc = bass.Bass()
    x_in = nc.declare_dram_parameter("x", [T_SHARD, N], bf16, isOutput=False)
    d_in = nc.declare_dram_parameter("drep", [P0, N], bf16, isOutput=False)
    out = nc.declare_dram_parameter("out", [T_SHARD, N], bf16, isOutput=True)
    warm = nc.dram_tensor("warm", [2, N], bf16)  # warm-up write target

    x_ap = x_in[:]
    o_ap = out[:]
    offs = np.cumsum([0] + tile_p)
    x_v = [x_ap[offs[i] : offs[i + 1]] for i in range(len(tile_p))]
    o_v = [o_ap[offs[i] : offs[i + 1]] for i in range(len(tile_p))]

    n_tiles = len(tile_p)
    H = N // 2
    h0, h1 = slice(0, H), slice(H, N)
    # (tile, col_slice) units in mul/store order: two column-halves per tile
    units = [(t, cs) for t in range(n_tiles) for cs in (h0, h1)]
    n_mul = len(units)                    # 18
    n_scalar_units = n_mul - n_sync_stores

    with ExitStack() as ctx:
        s_d0 = ctx.enter_context(nc.semaphore("s_d0"))
        s_d1 = ctx.enter_context(nc.semaphore("s_d1"))
        s_x0a = ctx.enter_context(nc.semaphore("s_x0a"))
        s_x0b = ctx.enter_context(nc.semaphore("s_x0b"))
        s_ld = [
            ctx.enter_context(nc.semaphore(f"s_ld{i}"))
            for i in range(1, n_tiles)
        ]  # s_ld[i-1] is tile i's load
        s_mul = ctx.enter_context(nc.semaphore("s_mul"))
        s_st = ctx.enter_context(nc.semaphore("s_st"))
        s_st2 = ctx.enter_context(nc.semaphore("s_st2"))
        s_warm = ctx.enter_context(nc.semaphore("s_warm"))

        db = ctx.enter_context(nc.sbuf_tensor("db", [P0, N], bf16))
        # dedicated never-written scratch: the t=0 warm-ups read it
        # (contents irrelevant; target is DRAM scratch)
        wsrc = ctx.enter_context(nc.sbuf_tensor("wsrc", [1, N], bf16))
        xts = [
            ctx.enter_context(nc.sbuf_tensor(f"xt{i}", [p, N], bf16))
            for i, p in enumerate(tile_p)
        ]

        with nc.Block() as block:

            @block.sync
            def _(sync):
                # column-split head: d half, x0 half, d half, x0 half —
                # the first multiply only needs the first half of each
                sync.dma_start(out=db[:, h0], in_=d_in[:, h0]).then_inc(s_d0, 16)
                sync.dma_start(out=xts[0][:, h0], in_=x_v[0][:, h0]).then_inc(
                    s_x0a, 16
                )
                sync.dma_start(out=db[:, h1], in_=d_in[:, h1]).then_inc(s_d1, 16)
                sync.dma_start(out=xts[0][:, h1], in_=x_v[0][:, h1]).then_inc(
                    s_x0b, 16
                )
                for i in range(1, n_tiles):
                    sync.dma_start(out=xts[i][:], in_=x_v[i]).then_inc(
                        s_ld[i - 1], 16
                    )
                sync.dma_start(out=warm[0, None, :], in_=wsrc[:]).then_inc(
                    s_warm, 16
                )
                # tail stores ride the sync ring: it is idle once the
                # loads drain, so the store backlog drains on both rings
                for k in range(n_scalar_units, n_mul):
                    t, cs = units[k]
                    sync.wait_ge(s_mul, k + 1)
                    sync.dma_start(out=o_v[t][:, cs], in_=xts[t][:, cs]).then_inc(
                        s_st2, 16
                    )
                sync.wait_ge(s_st2, 16 * n_sync_stores)
                sync.wait_ge(s_warm, 32)

            @block.scalar
            def _(scalar):
                # no-wait warm-up: the first DMA on the qAct ring pays a
                # ~4.5 us cold start, so burn it at t=0 on a dummy write
                # instead of on the first output store
                scalar.dma_start(out=warm[1, None, :], in_=wsrc[:]).then_inc(
                    s_warm, 16
                )
                for k in range(n_scalar_units):
                    t, cs = units[k]
                    scalar.wait_ge(s_mul, k + 1)
                    scalar.dma_start(
                        out=o_v[t][:, cs], in_=xts[t][:, cs]
                    ).then_inc(s_st, 16)
                scalar.wait_ge(s_st, 16 * n_scalar_units)
                scalar.wait_ge(s_warm, 32)

            @block.vector
            def _(vector):
                for k, (t, cs) in enumerate(units):
                    p = tile_p[t]
                    if k == 0:
                        vector.wait_ge(s_d0, 16)
                        vector.wait_ge(s_x0a, 16)
                    elif k == 1:
                        vector.wait_ge(s_d1, 16)
                        vector.wait_ge(s_x0b, 16)
                    elif t >= 1 and cs is h0:
                        vector.wait_ge(s_ld[t - 1], 16)
                    vector.tensor_mul(
                        out=xts[t][:, cs], in0=xts[t][:, cs], in1=db[:p, cs]
                    ).then_inc(s_mul, 1)

    nc.finalize()
    return nc


def _get_nc():
    global _CACHED_NC
    if _CACHED_NC is None:
        _CACHED_NC = _build_nc()
    return _CACHED_NC


def _shard_inputs(x, W):
    import ml_dtypes

    bf16 = ml_dtypes.bfloat16
    x = np.ascontiguousarray(np.asarray(x, dtype=np.float32)).astype(bf16)
    W = np.asarray(W, dtype=np.float32)
    d = np.ascontiguousarray(np.diagonal(W))
    d = np.where(np.abs(d) > THRESHOLD, d, np.float32(0.0)).astype(bf16)
    drep = np.ascontiguousarray(np.broadcast_to(d[None, :], (P0, N)))
    assert x.shape == (TOKENS, N) and drep.shape == (P0, N)
    return [
        {"x": x[c * T_SHARD : (c + 1) * T_SHARD], "drep": drep}
        for c in range(N_CORES)
    ]


def _run(x, W, **spmd_kwargs):
    from concourse.bass_utils import run_bass_kernel_spmd

    nc = _get_nc()
    in_maps = _shard_inputs(x, W)
    res = run_bass_kernel_spmd(nc, in_maps, list(range(N_CORES)), **spmd_kwargs)
    out = np.concatenate(
        [res.results[c]["out"] for c in range(N_CORES)], axis=0
    ).astype(np.float32)
    return out, res


def kernel(x, W):
    out, _ = _run(x, W)
    return out
